# revision 21
# baseline (speedup 1.0000x reference)
"""Trainium2 Bass kernel for nn_Grid_fun: out = tile(feat(z), 6) @ a.

Math: z = [x, 1] (N,4); feat = (z otimes z).reshape(N,16); out = tile(feat,6) @ a
    = feat @ a_eff  where a_eff = a.reshape(6,16,3).sum(0)   [16,3]
    => out[n,c] = z[n]^T A_c z[n],  A_c = a_eff[:,c].reshape(4,4)

Device algorithm (per core, data-parallel over N):
  Host stages x as Z2 [106, 4608] bf16: half h at partition base 64h holds
  rows 3g+j of z-cols m = 4608h + u (G=14 points per z-col, F=9216 z-cols).
  PE base partitions must be in {0,64} for K,M<=64 - hence the two-half
  layout (~83% DMA partition utilization, bf16 halves the line bytes).
  Per vps tile v (6 total; h=v%2, col-group jg=v//2):
    mm1 x3: V[127, 512i..] = pv^T @ Z2[64h:64h+42, 512(3jg+i)..]   (bf16)
    ACT:    R[127, 1536] = Square(V + bias)  (canonical/tailored square
            basis; R row 126 = (0+1)^2 = 1 carries the constant via ab)
    mm2 x3: block k=3v+i -> pps[k//2][64*(k%2) : +42] = ab^T @ R  (fp32r,
            2-stacked in PSUM at offsets {0,64})
  DVE copies pps -> bf16 SBUF (junk rows 42:64 ignored by host), gpsimd
  issues the output DMAs. PE warm-up matmuls fight the HAM clock gate.
"""

import sys

if "/opt/trn_rl_repo" not in sys.path:
    sys.path.insert(0, "/opt/trn_rl_repo")

from contextlib import ExitStack

import ml_dtypes
import numpy as np

import concourse.bass as bass
import concourse.mybir as mybir
import concourse.tile as tile
from concourse import bacc
from concourse.bass_utils import run_bass_kernel_spmd

N_CORES = 8
N_POINTS = 1_000_000
N_PER_CORE = N_POINTS // N_CORES  # 125000
G = 14  # points per z-column
F = 9216  # z-columns per core (18*512); G*F = 129024 >= N_PER_CORE
NPAD = G * F
HCOLS = F // 2  # 4608 z-cols per half
NV = 6  # vps tiles (3 blocks of 512 each)
CH = 512
N_WARM = 30  # PE warm-up matmuls (127 rows each, ~3.2us cold: flips HAM)
SQ_ACT = 1376  # square columns on ACT per 1536-tile; DVE squares the rest

_CACHE: dict = {}


def _build_nc():
    nc = bacc.Bacc("TRN2", target_bir_lowering=False)
    f32 = mybir.dt.float32
    bf16 = mybir.dt.bfloat16

    # All DMA partition counts are multiples of 16: the DGE splits one
    # DMA's descriptors into equal chunks over the largest divisor of
    # ndesc <= 16 SDMA engines (106 rows -> 2 engines; 112 -> 16).
    z_d = nc.dram_tensor("z", [112, HCOLS], bf16, kind="ExternalInput")
    pv_d = nc.dram_tensor("pv", [112, 127], bf16, kind="ExternalInput")
    ab_d = nc.dram_tensor("ab", [128, 42], bf16, kind="ExternalInput")
    o_d = nc.dram_tensor("o", [112, HCOLS], bf16, kind="ExternalOutput")
    sink_d = nc.dram_tensor("sink", [112, 2], f32, kind="ExternalOutput")

    with tile.TileContext(nc) as tc:
        with ExitStack() as ctx:
            cpool = ctx.enter_context(tc.tile_pool(name="consts", bufs=1))
            zpool = ctx.enter_context(tc.tile_pool(name="zt", bufs=2))
            rpool = ctx.enter_context(tc.tile_pool(name="rt", bufs=2))
            opool = ctx.enter_context(tc.tile_pool(name="ot", bufs=3))
            vpool = ctx.enter_context(
                tc.tile_pool(name="vps", bufs=2, space="PSUM")
            )
            ppool = ctx.enter_context(
                tc.tile_pool(name="pps", bufs=2, space="PSUM")
            )

            # PE warm-up: the HAM clock gate un-throttles 1.2 -> 2.4 GHz only
            # after ~3.4us of gap-free PE activity. Warm-up weights come from
            # a memset tile (no DMA dependency), so the PE starts as soon as
            # the engines boot - well before the first input chunk lands.
            wg = cpool.tile([42, 128], bf16)
            nc.vector.memset(wg[:], 0.0)
            warm = ppool.tile([127, CH], f32, tag="pps")
            for _ in range(N_WARM):
                nc.tensor.matmul(
                    warm[:, 0:127], wg[:, 0:127], wg[:, 0:127],
                    start=True, stop=True,
                )

            zts = []
            zt = zpool.tile([112, 3 * CH], bf16, name="zt0")
            nc.sync.dma_start(zt[:], z_d[:, 0 : 3 * CH])
            zts.append(zt)
            pv = cpool.tile([112, 127], bf16)
            nc.sync.dma_start(pv[:], pv_d[:, :])
            zt = zpool.tile([112, 3 * CH], bf16, name="zt1")
            nc.sync.dma_start(zt[:], z_d[:, 3 * CH : 6 * CH])
            zts.append(zt)
            ab = cpool.tile([128, 42], bf16)
            nc.sync.dma_start(ab[:], ab_d[:, :])

            # ACT table warm-up (Square table load ~1.3us) off critical path.
            wsq = cpool.tile([112, 2], f32)
            nc.scalar.activation(
                wsq[0:42, 0:1], wg[:, 0:1],
                mybir.ActivationFunctionType.Square,
            )
            # Keep the warm-ups live (read one column into the sink output).
            nc.vector.tensor_copy(wsq[0:112, 1:2], warm[0:112, 0:1])
            nc.sync.dma_start(sink_d[:, :], wsq[:])

            pps_tiles = {}

            def mm2_stage(v, rt):
                # deferred one tile: the PE queue is in-order, so mm2(v)
                # emitted before mm1(v+1) would stall the PE on the square.
                for i in range(3):
                    k = 3 * v + i
                    p, s = k // 2, k % 2
                    if s == 0:
                        pps_tiles[p] = ppool.tile(
                            [106, CH], f32, tag="pps", name=f"pps{p}"
                        )
                    nc.tensor.matmul(
                        pps_tiles[p][64 * s : 64 * s + 42, :],
                        ab[0:127, :],
                        rt[:, i * CH : (i + 1) * CH],
                        start=True,
                        stop=True,
                    )
                    if s == 1:
                        ot = opool.tile([112, CH], bf16, name=f"ot{p}")
                        nc.vector.tensor_copy(ot[0:106, :], pps_tiles[p][:])
                        nc.sync.dma_start(
                            o_d[:, CH * p : CH * (p + 1)], ot[:]
                        )

            pending = None
            for v in range(NV):
                h, jg = v % 2, v // 2
                if h == 0 and jg == 1:
                    # prefetch the last input chunk as its buffer frees
                    zt2 = zpool.tile([112, 3 * CH], bf16, name="zt2")
                    nc.sync.dma_start(zt2[:], z_d[:, 6 * CH : 9 * CH])
                    zts.append(zt2)
                zt = zts[jg]

                vps = vpool.tile([127, 3 * CH], f32)
                for i in range(3):
                    nc.tensor.matmul(
                        vps[:, i * CH : (i + 1) * CH],
                        pv[64 * h : 64 * h + 43, :],
                        zt[64 * h : 64 * h + 43, i * CH : (i + 1) * CH],
                        start=True,
                        stop=True,
                    )

                rt = rpool.tile([127, 3 * CH], bf16)
                if v == 0 or v == NV - 1:
                    # split first/last squares: v0 so the PE's pipeline-fill
                    # gap stays short (a long idle re-throttles HAM), v5 so
                    # the tail mm2s start sooner
                    for i in range(3):
                        nc.scalar.activation(
                            rt[:, i * CH : (i + 1) * CH],
                            vps[:, i * CH : (i + 1) * CH],
                            mybir.ActivationFunctionType.Square,
                        )
                else:
                    nc.scalar.activation(
                        rt[:], vps[:], mybir.ActivationFunctionType.Square,
                    )
                if pending is not None:
                    mm2_stage(*pending)
                pending = (v, rt)
            mm2_stage(*pending)
    nc.compile()
    return nc


def _coeffs(a: np.ndarray):
    """Host-side prep of the constant matrices from param a [96,3]."""
    a_eff = a.reshape(6, 16, 3).sum(0)  # [16,3]
    A = a_eff.T.reshape(3, 4, 4)  # A[c] with out_c = z^T A_c z
    As = 0.5 * (A + A.transpose(0, 2, 1))  # symmetrize
    Q = As[:, :3, :3]  # [3,3,3] quadratic part
    L = 2.0 * As[:, :3, 3]  # [3,3] linear coefs
    K = As[:, 3, 3].copy()  # [3] constants
    # guard tiny K (u_c = L_c / (2 K_c)); shift the constant via kconst fold
    Ksafe = np.where(np.abs(K) < 1e-3, 1.0, K)
    U = L / (2.0 * Ksafe[:, None])  # [3,3] tailored directions

    # basis quadratic parts: M[s] (3x3 sym) for s=0..8
    E = np.eye(3, dtype=np.float64)
    dirs = [
        (E[0], E[0]), (E[1], E[1]), (E[2], E[2]),
        (E[0] + E[1], E[0] + E[1]),
        (E[0] + E[2], E[0] + E[2]),
        (E[1] + E[2], E[1] + E[2]),
    ]
    M = np.zeros((9, 3, 3))
    for s, (u, v) in enumerate(dirs):
        M[s] = np.outer(u, v)
    for c in range(3):
        M[6 + c] = np.outer(U[c], U[c])

    def sym6(S):
        return np.array(
            [S[0, 0], S[1, 1], S[2, 2], S[0, 1] + S[1, 0],
             S[0, 2] + S[2, 0], S[1, 2] + S[2, 1]]
        )

    B6 = np.stack([sym6(M[s]) for s in range(9)])  # [9,6]
    W = np.zeros((3, 9))
    for c in range(3):
        rhs = sym6(Q[c]) - Ksafe[c] * B6[6 + c]
        W[c, :6] = np.linalg.solve(B6[:6].T, rhs)
        W[c, 6 + c] = Ksafe[c]
    kconst = K - Ksafe
    return U, W, kconst


def _host_tensors(a: np.ndarray):
    """pv [112,127] bf16, ab [128,42] bf16.

    mm1 column layout (M=127): col 9g+s = form s of group g; col 126 is the
    constant generator (V = 1 via the ones-row -> R = 1). The z halves carry
    a ones-row (row 42 of each half), so former activation biases live in
    pv row 42. pv rows duplicated at partition bases 0 and 64.
    """
    U, W, kconst = _coeffs(a.astype(np.float64))
    pv1 = np.zeros((43, 127), dtype=np.float32)
    ab = np.zeros((128, 42), dtype=np.float32)
    forms = [
        [(0, 1.0)], [(1, 1.0)], [(2, 1.0)],
        [(0, 1.0), (1, 1.0)], [(0, 1.0), (2, 1.0)], [(1, 1.0), (2, 1.0)],
    ]
    for g in range(G):
        for s in range(9):
            col = 9 * g + s
            if s < 6:
                for j, v in forms[s]:
                    pv1[3 * g + j, col] = v
            else:
                c = s - 6
                for j in range(3):
                    pv1[3 * g + j, col] = U[c, j]
                pv1[42, col] = 1.0
        for c in range(3):
            orow = 3 * g + c
            for s in range(9):
                ab[9 * g + s, orow] = W[c, s]
            ab[126, orow] = kconst[c]
    pv1[42, 126] = 1.0
    pv = np.zeros((112, 127), dtype=np.float32)
    pv[0:43] = pv1
    pv[64:107] = pv1
    return pv.astype(ml_dtypes.bfloat16), ab.astype(ml_dtypes.bfloat16)


def _pack_x(x_core: np.ndarray) -> np.ndarray:
    """[N_PER_CORE, 3] f32 -> Z2 [112, 4608] bf16 (device input layout)."""
    xp = np.zeros((NPAD, 3), dtype=np.float32)
    xp[:N_PER_CORE] = x_core
    z = xp.reshape(F, G, 3).transpose(1, 2, 0).reshape(42, F)
    z2 = np.zeros((112, HCOLS), dtype=np.float32)
    z2[0:42] = z[:, :HCOLS]
    z2[42] = 1.0
    z2[64:106] = z[:, HCOLS:]
    z2[106] = 1.0
    return np.ascontiguousarray(z2.astype(ml_dtypes.bfloat16))


def _unpack_o(o: np.ndarray) -> np.ndarray:
    """o [112, 4608] bf16 -> [N_PER_CORE, 3] f32."""
    of = np.asarray(o, dtype=np.float32)
    full = np.empty((NPAD, 3), dtype=np.float32)
    # block k: rows 64*(k%2)+3g+cc, cols 512*(k//2)+u of o hold point
    # p = 14*m+g, m = 4608*h + 512*j + u, h = (k//3)%2, j = 3*(k//6)+k%3
    for k in range(18):
        v, i = k // 3, k % 3
        h, j = v % 2, 3 * (v // 2) + i
        p, s = k // 2, k % 2
        blk = of[64 * s : 64 * s + 42, CH * p : CH * (p + 1)]  # [3g+cc, u]
        m0 = 4608 * h + 512 * j
        full[G * m0 : G * (m0 + CH)] = (
            blk.reshape(G, 3, CH).transpose(2, 0, 1).reshape(G * CH, 3)
        )
    return full[:N_PER_CORE]


def kernel(x: np.ndarray, a: np.ndarray) -> np.ndarray:
    x = np.ascontiguousarray(x, dtype=np.float32)
    a = np.ascontiguousarray(a, dtype=np.float32)
    if "nc" not in _CACHE:
        _CACHE["nc"] = _build_nc()
    nc = _CACHE["nc"]

    pv, ab = _host_tensors(a)
    in_maps = []
    for ci in range(N_CORES):
        z2 = _pack_x(x[ci * N_PER_CORE : (ci + 1) * N_PER_CORE])
        in_maps.append({"z": z2, "pv": pv, "ab": ab})

    res = run_bass_kernel_spmd(nc, in_maps, list(range(N_CORES)))

    out = np.empty((N_POINTS, 3), dtype=np.float32)
    for ci in range(N_CORES):
        out[ci * N_PER_CORE : (ci + 1) * N_PER_CORE] = _unpack_o(
            res.results[ci]["o"]
        )
    return out
